# revision 52
# baseline (speedup 1.0000x reference)
"""Trainium2 Bass kernel for the CoSSM block (bidirectional Mamba on two streams).

Sharding: 8 cores = 2 streams x 4 batches; each core runs the full pipeline for
its (stream, batch) slice.  Weights are replicated; the per-core input map
carries the stream-specific resbranch/LN weights.

v2: fp16 matmuls throughout; selective scan packed as two [128, 8*513]
parity mega-tiles per (dc, branch) (8 n-states + reset columns each).
Odd decay powers exp(-(2j+1)*delta) come from the scalar engine, even powers
from one DVE multiply by the broadcast E1 segment.  dBu/C-multiply/folds are
single wide DVE instructions; fold tail + small elementwise work runs on
GpSimd to keep DVE scan-dominated.
"""

import os
import sys
import numpy as np

for _p in ("/opt/trn_rl_repo",):
    if _p not in sys.path:
        sys.path.append(_p)

import concourse.mybir as mybir
from concourse import bacc
from concourse.tile import TileContext
from concourse.bass_utils import run_bass_kernel_spmd

F32 = mybir.dt.float32
F16 = mybir.dt.float16
F32R = mybir.dt.float32r
AL = mybir.AluOpType
AF = mybir.ActivationFunctionType
AX = mybir.AxisListType

B, L = 4, 512
D_IN, D_MODEL = 256, 1024
D_INNER, D_STATE, D_CONV, DT_RANK = 2048, 16, 4, 64
BN_EPS, LN_EPS = 1e-5, 1e-6

NDC = D_INNER // 128          # 16 channel chunks of d_inner
NMC = D_MODEL // 128          # 8 channel chunks of d_model
SEGW = L + 1                  # segment width incl. leading reset column
PAR = D_STATE // 2            # 8 segments per parity tile
EW = PAR * SEGW               # 4104

_CACHE = {}


def _build_program():
    nc = bacc.Bacc("TRN2", target_bir_lowering=False, debug=False)

    def din(name, shape, dt=F32):
        return nc.dram_tensor(name, list(shape), dt, kind="ExternalInput")

    d_x = din("x", [4, 128, D_IN], F32R)
    d_identr = din("identr", [128, 128], F32R)
    d_identf = din("identf", [128, 128], F16)
    d_ones = din("ones", [1, 128], F32R)
    d_onescol = din("onescol", [128, 1], F16)
    d_rbAll = din("rbAll", [128, NMC, 8, 128], F16)

    d_wproj = din("wproj", [8, 128, 2 * D_INNER], F16)
    d_convd = din("convd", [2, NDC, 128, 4 * 128], F16)

    d_xpw = din("xpw", [2, NDC, 128, 112], F16)
    d_dtw = din("dtw", [2, 64, D_INNER], F16)



    d_opw = din("opw", [NDC, 128, D_MODEL], F16)
    d_cpack = din("cpack", [128, 640])
    d_sel = din("sel", [D_STATE, D_STATE * 128], F16)

    d_out = nc.dram_tensor("y_out", [L, D_MODEL], F32, kind="ExternalOutput")
    d_x1 = nc.dram_tensor("x1_spill", [NMC, 128, L], F32)
    d_z = nc.dram_tensor("z_spill", [NDC, 128, L], F16)
    d_d8 = [nc.dram_tensor(f"d8_spill{br}", [NDC, 128, L], F16) for br in range(2)]

    with TileContext(nc) as tc:
        with tc.tile_pool(name="sb", bufs=1) as sb, \
             tc.tile_pool(name="wp", bufs=3) as wp, \
             tc.tile_pool(name="tp", bufs=2) as tp, \
             tc.tile_pool(name="ps", bufs=5, space="PSUM") as ps, \
             tc.tile_pool(name="px", bufs=1, space="PSUM") as pxp, \
             tc.tile_pool(name="st", bufs=1, space="PSUM") as stp:

            def psum(shape=(128, 512)):
                return ps.tile(list(shape), F32, tag="mm", name="mm")

            # ---- constants -------------------------------------------------
            t_idr = sb.tile([128, 128], F32R, tag="idr", name="idr")
            nc.sync.dma_start(out=t_idr[:], in_=d_identr[:])
            t_idf = sb.tile([128, 128], F16, tag="idf", name="idf")
            nc.sync.dma_start(out=t_idf[:], in_=d_identf[:])
            t_ones = sb.tile([1, 128], F32R, tag="ones", name="ones")
            nc.sync.dma_start(out=t_ones[:], in_=d_ones[:])
            t_onescol = sb.tile([128, 1], F16, tag="onescol", name="onescol")
            nc.sync.dma_start(out=t_onescol[:], in_=d_onescol[:])
            t_cpk = sb.tile([128, 640], F32, tag="cpk", name="cpk")
            nc.sync.dma_start(out=t_cpk[:], in_=d_cpack[:])
            t_sel = sb.tile([D_STATE, D_STATE * 128], F16, tag="sel", name="sel")
            nc.sync.dma_start(out=t_sel[:], in_=d_sel[:])

            # ---- phase A: load x, transpose to [c, l] fp16 -----------------
            t_xpad = [sb.tile([128, 514], F16, tag=f"xp{cc}", name=f"xp{cc}") for cc in range(2)]
            for cc in range(2):
                nc.vector.memset(t_xpad[cc][:].bitcast(F32), 0.0)
            for i in range(4):
                xt = tp.tile([128, D_IN], F32R, tag="xin", name="xin", bufs=1)
                nc.sync.dma_start(out=xt[:], in_=d_x[i])
                for cc in range(2):
                    pt = ps.tile([128, 128], F32R, tag="mm", name="mm")
                    nc.tensor.transpose(pt[:], xt[:, cc * 128:(cc + 1) * 128], t_idr[:])
                    nc.scalar.copy(t_xpad[cc][:, 1 + i * 128:1 + (i + 1) * 128], pt[:])

            # ---- phase B: resbranch -> x1 (f32 spill + fp16 mega) ----------
            t_x1m = sb.tile([128, NMC * L], F16, tag="x1m", name="x1m")
            for m in range(NMC):
                wt = wp.tile([128, 8, 128], F16, tag="wB", name="wB", bufs=2)
                nc.gpsimd.dma_start(out=wt[:], in_=d_rbAll[:, m])
                pc = psum()
                for cc in range(2):
                    nc.tensor.matmul(pc[:], wt[:, 6 + cc, :], t_xpad[cc][:, 1:513],
                                     start=(cc == 0), stop=(cc == 1))
                pk = psum()
                idx = 0
                for k in range(3):
                    for cc in range(2):
                        nc.tensor.matmul(pk[:], wt[:, k * 2 + cc, :], t_xpad[cc][:, k:k + 512],
                                         start=(idx == 0), stop=(idx == 5))
                        idx += 1
                tr = tp.tile([128, L], F32, tag="sA", name="sA", bufs=1)
                nc.scalar.activation(tr[:], pk[:], AF.Relu, bias=t_cpk[:, 608 + m:609 + m])
                x1f = tp.tile([128, L], F32, tag="x1f", name="x1f", bufs=1)
                nc.vector.tensor_tensor(out=x1f[:], in0=tr[:], in1=pc[:], op=AL.add)
                nc.sync.dma_start(out=d_x1[m], in_=x1f[:])
                nc.scalar.activation(t_x1m[:, m * L:(m + 1) * L], x1f[:], AF.Copy)

            # ---- phase E tiles (persistent, parity-packed) -----------------
            t_da = sb.tile([128, 2 * EW], F16, tag="da", name="da")
            t_dbu = sb.tile([128, 2 * EW], F16, tag="dbu", name="dbu")
            t_hh = sb.tile([128, 2 * EW], F16, tag="hh", name="hh")
            t_BbM = sb.tile([128, 2 * EW], F16, tag="BbM", name="BbM")
            t_CbM = sb.tile([128, 2 * EW], F16, tag="CbM", name="CbM")
            for t in (t_da, t_dbu, t_BbM, t_CbM):
                nc.vector.memset(t[:].bitcast(F32), 0.0)

            t_y = [sb.tile([128, L], F16, tag=f"y{dc}", name=f"y{dc}") for dc in range(NDC)]
            t_u = [[None] * NDC, [None] * NDC]
            t_xi = [sb.tile([128, 518], F16, tag=f"xi{dc}", name=f"xi{dc}") for dc in range(NDC)]
            for dc in range(NDC):
                nc.vector.memset(t_xi[dc][:].bitcast(F32), 0.0)

            def inproj_group(g):
                pts = [psum() for _ in range(4)]
                for kc in range(NMC):
                    wt = wp.tile([128, 512], F16, tag="wbig", name="wbig", bufs=6)
                    q = nc.gpsimd if kc % 2 == 0 else nc.sync
                    q.dma_start(out=wt[:], in_=d_wproj[kc][:, g * 512:(g + 1) * 512])
                    for mj in range(4):
                        nc.tensor.matmul(pts[mj][:], wt[:, mj * 128:(mj + 1) * 128],
                                         t_x1m[:, kc * L:(kc + 1) * L],
                                         start=(kc == 0), stop=(kc == NMC - 1))
                for mj in range(4):
                    mm = g * 4 + mj
                    if mm < NDC:
                        nc.scalar.copy(t_xi[mm][:, 3:515], pts[mj][:])
                    else:
                        zt = tp.tile([128, L], F16, tag="ztmp", name="ztmp")
                        nc.scalar.copy(zt[:], pts[mj][:])
                        nc.sync.dma_start(out=d_z[mm - NDC], in_=zt[:])

            def conv_u(br, dc):
                cdt = wp.tile([128, 512], F16, tag="wbig", name="wbig", bufs=6)
                q = nc.gpsimd if dc % 2 == 0 else nc.sync
                q.dma_start(out=cdt[:], in_=d_convd[br, dc])
                pu = psum()
                for k in range(4):
                    if br == 0:
                        rhs = t_xi[dc][:, k:k + 512]
                    else:
                        rhs = t_xi[dc][:, 6 - k:518 - k][:, ::-1]
                    nc.tensor.matmul(pu[:], cdt[:, k * 128:(k + 1) * 128], rhs,
                                     start=(k == 0), stop=(k == 3))
                ut = sb.tile([128, L], F16, tag=f"u_{dc}", name=f"u{br}_{dc}")
                nc.scalar.activation(ut[:], pu[:], AF.Silu, bias=t_cpk[:, dc * 2 + br:dc * 2 + br + 1])
                t_u[br][dc] = ut
                if br == 0:
                    nc.scalar.activation(t_y[dc][:], ut[:], AF.Copy, scale=t_cpk[:, 64 + dc * 2:65 + dc * 2])
                else:
                    yb = tp.tile([128, L], F16, tag="ztmp", name="yb")
                    nc.scalar.activation(yb[:], ut[:], AF.Copy, scale=t_cpk[:, 65 + dc * 2:66 + dc * 2])
                    nc.vector.tensor_tensor(out=t_y[dc][:], in0=t_y[dc][:], in1=yb[:, ::-1], op=AL.add)

            def e_head(br):
                # x_proj accumulation over dc -> px [112, 512]
                px = pxp.tile([112, 512], F32, tag="px", name="px")
                for dc in range(NDC):
                    wt = wp.tile([128, 112], F16, tag="wxp", name="wxp")
                    nc.gpsimd.dma_start(out=wt[:], in_=d_xpw[br, dc])
                    nc.tensor.matmul(px[:], wt[:], t_u[br][dc][:],
                                     start=(dc == 0), stop=(dc == NDC - 1))
                t_dtT = sb.tile([64, L], F16, tag="dtT", name="dtT")
                nc.scalar.copy(t_dtT[:], px[0:64, :])
                t_Brow = sb.tile([D_STATE, L], F16, tag="Brow", name="Brow")
                nc.scalar.copy(t_Brow[:], px[64:80, :])
                t_Crow = sb.tile([D_STATE, L], F16, tag="Crow", name="Crow")
                nc.scalar.copy(t_Crow[:], px[96:112, :])

                # delta prepass in blocks of 4; block 0 inline, rest deferred
                def prepass_blk(blk):
                    def run():
                        dcs = list(range(4 * blk, 4 * blk + 4))
                        ets = []
                        for dc in dcs:
                            wdt = wp.tile([64, 128], F16, tag="wdt", name="wdt")
                            nc.gpsimd.dma_start(out=wdt[:], in_=d_dtw[br][:, dc * 128:(dc + 1) * 128])
                            pd = psum()
                            nc.tensor.matmul(pd[:], wdt[:], t_dtT[:], start=True, stop=True)
                            ete = tp.tile([128, L], F16, tag="ete", name="ete", bufs=4)
                            nc.scalar.activation(ete[:], pd[:], AF.Exp, bias=t_cpk[:, 32 + dc * 2 + br:33 + dc * 2 + br])
                            ets.append(ete)
                        for i, dc in enumerate(dcs):
                            d8 = tp.tile([128, L], F16, tag="d8", name="d8")
                            nc.scalar.activation(d8[:], ets[i][:], AF.Ln, bias=1.0)
                            nc.sync.dma_start(out=d_d8[br][dc], in_=d8[:])
                            ut = t_u[br][dc]
                            nc.gpsimd.tensor_tensor(out=ut[:], in0=d8[:], in1=ut[:], op=AL.mult)
                    return run
                prepass_blk(0)()
                return (t_Brow, t_Crow), [prepass_blk(b) for b in range(1, 4)]

            def e_bcasts(br, t_Brow, t_Crow):
                # broadcast B/C rows into parity-packed tiles
                for p in range(2):
                    for j in range(PAR):
                        n0 = 2 * j + p
                        selap = t_sel[:, n0 * 128:(n0 + 1) * 128]
                        off = p * EW + j * SEGW
                        pb = psum()
                        nc.tensor.matmul(pb[:], selap, t_Brow[:], start=True, stop=True)
                        nc.vector.tensor_copy(t_BbM[:, off + 1:off + SEGW], pb[:])
                        pcb = psum()
                        nc.tensor.matmul(pcb[:], selap, t_Crow[:], start=True, stop=True)
                        nc.vector.tensor_copy(t_CbM[:, off + 1:off + SEGW], pcb[:])

            pend = {}

            def e_finalize(br, pos0):
                dcp, pfp = pend.pop("v")
                if br == 0:
                    nc.vector.tensor_tensor(out=t_y[dcp][:], in0=t_y[dcp][:], in1=pfp[:], op=AL.add)
                else:
                    nc.vector.tensor_tensor(out=t_y[dcp][:], in0=t_y[dcp][:],
                                            in1=pfp[:][:, ::-1], op=AL.add)
                    nc.vector.tensor_tensor(out=t_y[dcp][:], in0=t_y[dcp][:],
                                            in1=t_szt[dcp][:, 0:512], op=AL.mult)
                    wt = wp.tile([128, 512], F16, tag="wbig", name="wbig", bufs=6)
                    q = nc.gpsimd if dcp % 2 == 0 else nc.sync
                    q.dma_start(out=wt[:], in_=d_opw[dcp][:, 0:512])
                    for mj in range(4):
                        nc.tensor.matmul(pos0[mj][:], wt[:, mj * 128:(mj + 1) * 128],
                                         t_y[dcp][:], start=(dcp == 0), stop=(dcp == NDC - 1))

            def e_loop(br, dcs, pos0=None, hooks=None, flush=True):
                dbu_d = t_dbu[:].rearrange("q (n l) -> q n l", n=2 * PAR)[:, :, 1:SEGW]
                Bb_d = t_BbM[:].rearrange("q (n l) -> q n l", n=2 * PAR)[:, :, 1:SEGW]

                for dc in dcs:
                    d8l = tp.tile([128, L], F16, tag="d8l", name="d8l", bufs=3)
                    nc.sync.dma_start(out=d8l[:], in_=d_d8[br][dc])
                    # all 16 decay powers exp(-(n0+1) delta) on scalar engine
                    for n0 in range(D_STATE):
                        off = (n0 % 2) * EW + (n0 // 2) * SEGW
                        nc.scalar.activation(
                            t_da[:, off + 1:off + SEGW], d8l[:],
                            AF.Exp, scale=t_cpk[:, 96 + br * 256 + dc * 16 + n0:97 + br * 256 + dc * 16 + n0])
                    # dBu mega-mult (w lives in the u tile)
                    w16b = t_u[br][dc][:, None, :].broadcast_to([128, 2 * PAR, L])
                    nc.vector.tensor_tensor(out=dbu_d, in0=w16b, in1=Bb_d, op=AL.mult)
                    # previous dc's fold result lands while this scan runs
                    if "v" in pend:
                        e_finalize(br, pos0)
                    # scan
                    nc.vector.tensor_tensor_scan(t_hh[:], t_da[:], t_dbu[:], 0.0, AL.mult, AL.add)
                    # C multiply in place, split so the PE fold starts early
                    zv = t_hh[:].rearrange("q (n l) -> q n l", n=2 * PAR)
                    pf = psum()
                    nc.vector.tensor_tensor(out=t_hh[:, 0:EW], in0=t_hh[:, 0:EW],
                                            in1=t_CbM[:, 0:EW], op=AL.mult)
                    for n in range(PAR):
                        nc.tensor.matmul(pf[:], t_idf[:], zv[:, n, 1:SEGW],
                                         start=(n == 0), stop=False)
                    nc.vector.tensor_tensor(out=t_hh[:, EW:2 * EW], in0=t_hh[:, EW:2 * EW],
                                            in1=t_CbM[:, EW:2 * EW], op=AL.mult)
                    for n in range(PAR, 2 * PAR):
                        nc.tensor.matmul(pf[:], t_idf[:], zv[:, n, 1:SEGW],
                                         start=False, stop=(n == 2 * PAR - 1))
                    pend["v"] = (dc, pf)
                    if hooks and dc in hooks:
                        hooks[dc]()
                if flush and "v" in pend:
                    e_finalize(br, pos0)

            # interleave: xi-producing in_proj groups, conv0 as chunks arrive
            for g in range(4):
                inproj_group(g)
                for dc in range(4 * g, 4 * g + 4):
                    conv_u(0, dc)
            rows0, blks0 = e_head(0)
            e_bcasts(0, *rows0)
            e_loop(0, range(0, 8), hooks={2: lambda: [b() for b in blks0]}, flush=False)
            # branch-1 prep and z-projection run under branch-0's scan stream
            for g in range(4, 8):
                inproj_group(g)
            t_szt = [None] * NDC
            for dc in range(0, 8):
                conv_u(1, dc)
            for dc in range(0, 8):
                zt = tp.tile([128, L], F16, tag="ztmp", name="ztmp")
                nc.sync.dma_start(out=zt[:], in_=d_z[dc])
                t_szt[dc] = sb.tile([128, 518], F16, tag=f"xi{dc}", name=f"szt{dc}")
                nc.scalar.activation(t_szt[dc][:, 0:512], zt[:], AF.Silu)
            e_loop(0, range(8, NDC))
            for dc in range(8, NDC):
                conv_u(1, dc)
            for dc in range(8, NDC):
                zt = tp.tile([128, L], F16, tag="ztmp", name="ztmp")
                nc.sync.dma_start(out=zt[:], in_=d_z[dc])
                t_szt[dc] = sb.tile([128, 518], F16, tag=f"xi{dc}", name=f"szt{dc}")
                nc.scalar.activation(t_szt[dc][:, 0:512], zt[:], AF.Silu)
            rows1, blks1 = e_head(1)
            e_bcasts(1, *rows1)
            pos0 = [psum() for _ in range(4)]
            e_loop(1, range(NDC), pos0=pos0, hooks={2: lambda: [b() for b in blks1]})

            # ---- phase F: out_proj second half, layernorm, residual --------
            t_o1 = sb.tile([128, 2 * EW], F16, tag="da", name="o1")
            for mj in range(4):
                nc.scalar.copy(t_o1[:, mj * L:(mj + 1) * L], pos0[mj][:])
            pos1 = [psum() for _ in range(4)]
            for dc in range(NDC):
                wt = wp.tile([128, 512], F16, tag="wbig", name="wbig", bufs=6)
                q = nc.gpsimd if dc % 2 == 0 else nc.sync
                q.dma_start(out=wt[:], in_=d_opw[dc][:, 512:1024])
                for mj in range(4):
                    nc.tensor.matmul(pos1[mj][:], wt[:, mj * 128:(mj + 1) * 128], t_y[dc][:],
                                     start=(dc == 0), stop=(dc == NDC - 1))
            for mj in range(4):
                nc.scalar.copy(t_o1[:, (4 + mj) * L:(5 + mj) * L], pos1[mj][:])

            # layernorm stats via column-sum matmuls
            pm = stp.tile([1, 512], F32, tag="stm", name="stm")
            for m in range(NMC):
                nc.tensor.matmul(pm[:], t_onescol[:], t_o1[:, m * L:(m + 1) * L],
                                 start=(m == 0), stop=(m == NMC - 1))
            pq = stp.tile([1, 512], F32, tag="stq", name="stq")
            for m in range(NMC):
                sq = tp.tile([128, L], F16, tag="sB", name="sB")
                nc.scalar.activation(sq[:], t_o1[:, m * L:(m + 1) * L], AF.Square)
                nc.tensor.matmul(pq[:], t_onescol[:], sq[:],
                                 start=(m == 0), stop=(m == NMC - 1))
            t_mean = sb.tile([1, L], F32R, tag="mean", name="mean")
            nc.scalar.activation(t_mean[:], pm[:], AF.Copy, scale=1.0 / D_MODEL)
            t_var = tp.tile([1, L], F32, tag="stat", name="stat")
            nc.scalar.activation(t_var[:], pq[:], AF.Copy, scale=1.0 / D_MODEL)
            msq = tp.tile([1, L], F32, tag="stat", name="stat")
            nc.vector.tensor_tensor(out=msq[:], in0=t_mean[:], in1=t_mean[:], op=AL.mult)
            nc.vector.tensor_tensor(out=t_var[:], in0=t_var[:], in1=msq[:], op=AL.subtract)
            t_eps = sb.tile([1, 1], F32, tag="eps", name="eps")
            nc.vector.memset(t_eps[:], LN_EPS)
            t_sd = tp.tile([1, L], F32, tag="stat2", name="stat2", bufs=1)
            nc.scalar.activation(t_sd[:], t_var[:], AF.Sqrt, bias=t_eps[:])
            t_isd = sb.tile([1, L], F32R, tag="isd", name="isd")
            with nc.allow_low_precision(reason="isd is a broadcast-matmul rhs"):
                nc.vector.reciprocal(out=t_isd[:], in_=t_sd[:])
            pmb = psum()
            nc.tensor.matmul(pmb[:], t_ones[:], t_mean[:], start=True, stop=True)
            t_mb = sb.tile([128, L], F16, tag="mb", name="mb")
            nc.scalar.copy(t_mb[:], pmb[:])
            pib = psum()
            nc.tensor.matmul(pib[:], t_ones[:], t_isd[:], start=True, stop=True)
            t_ib = sb.tile([128, L], F16, tag="ib", name="ib")
            nc.scalar.copy(t_ib[:], pib[:])

            t_of = []
            for m in range(NMC):
                x1r = tp.tile([128, L], F32, tag="sA", name="sA", bufs=1)
                nc.sync.dma_start(out=x1r[:], in_=d_x1[m])
                tt = tp.tile([128, L], F32, tag="sB2", name="sB2")
                nc.vector.tensor_tensor(out=tt[:], in0=t_o1[:, m * L:(m + 1) * L],
                                        in1=t_mb[:], op=AL.subtract)
                nc.vector.tensor_tensor(out=tt[:], in0=tt[:], in1=t_ib[:], op=AL.mult)
                nc.vector.tensor_scalar(out=tt[:], in0=tt[:],
                                        scalar1=t_cpk[:, 616 + m:617 + m], scalar2=t_cpk[:, 624 + m:625 + m],
                                        op0=AL.mult, op1=AL.add)
                ot = sb.tile([128, L], F16, tag=f"y{m}", name=f"of{m}")
                nc.vector.tensor_tensor(out=ot[:], in0=tt[:], in1=x1r[:], op=AL.add)
                t_of.append(ot)

            for i in range(4):
                outt = wp.tile([128, D_MODEL], F32, tag="outt", name="outt", bufs=2)
                for m in range(NMC):
                    ptr = ps.tile([128, 128], F16, tag="mm", name="mm")
                    nc.tensor.transpose(ptr[:], t_of[m][:, i * 128:(i + 1) * 128], t_idf[:])
                    nc.scalar.copy(outt[:, m * 128:(m + 1) * 128], ptr[:])
                nc.sync.dma_start(out=d_out[i * 128:(i + 1) * 128, :], in_=outt[:])

    nc.compile()
    return nc


def _prep_core_inputs(x, rb_conv_w, rb_bn_g, rb_bn_b, rb_skip_w, inp, ln_g, ln_b):
    f32, f16 = np.float32, np.float16
    out = {}
    out["x"] = np.ascontiguousarray(x.reshape(4, 128, D_IN)).astype(f32)
    out["identr"] = np.eye(128, dtype=f32)
    out["identf"] = np.eye(128, dtype=f16)
    out["ones"] = np.ones((1, 128), f32)
    out["onescol"] = np.ones((128, 1), f16)
    s = f32(1.0) / np.sqrt(np.float64(1.0 + BN_EPS))
    Wc = (rb_conv_w * (rb_bn_g * s)[:, None, None]).astype(f32)   # [1024,256,3]
    rbw = np.transpose(Wc, (2, 1, 0)).reshape(3, 2, 128, NMC, 128)  # [k,cc,p,m,out]
    rbs = rb_skip_w[:, :, 0].T.reshape(2, 128, NMC, 128)            # [cc,p,m,out]
    rbAll = np.zeros((128, NMC, 8, 128), f32)
    for k in range(3):
        for cc in range(2):
            rbAll[:, :, k * 2 + cc, :] = rbw[k, cc]
    for cc in range(2):
        rbAll[:, :, 6 + cc, :] = rbs[cc]
    out["rbAll"] = np.ascontiguousarray(rbAll).astype(f16)
    cpack = np.zeros((128, 640), f32)
    cpack[:, 608:616] = rb_bn_b.reshape(NMC, 128).T
    out["wproj"] = np.ascontiguousarray(inp["in_proj_w"].T.reshape(8, 128, 2 * D_INNER)).astype(f16)
    convd = np.zeros((2, NDC, 128, 4, 128), f32)
    r = np.arange(128)
    for br, key in enumerate(["conv_w_f", "conv_w_b"]):
        cw = inp[key].astype(f32)
        for dc in range(NDC):
            for k in range(4):
                convd[br, dc, r, k, r] = cw[dc * 128:(dc + 1) * 128, k]
    out["convd"] = convd.reshape(2, NDC, 128, 512).astype(f16)
    cb = np.stack([inp["conv_bias_f"].reshape(NDC, 128), inp["conv_bias_b"].reshape(NDC, 128)], -1)
    cpack[:, 0:32] = cb.transpose(1, 0, 2).reshape(128, 32)
    xpw = np.zeros((2, D_INNER, 112), np.float32)
    for br, key in enumerate(["x_proj_f", "x_proj_b"]):
        xp = inp[key].T  # [2048, 96]
        xpw[br, :, 0:80] = xp[:, 0:80]
        xpw[br, :, 96:112] = xp[:, 80:96]
    out["xpw"] = np.ascontiguousarray(xpw.reshape(2, NDC, 128, 112)).astype(f16)
    out["dtw"] = np.ascontiguousarray(np.stack(
        [inp["dt_w_f"].T, inp["dt_w_b"].T])).astype(f16)
    dtb = np.stack([inp["dt_bias_f"].reshape(NDC, 128), inp["dt_bias_b"].reshape(NDC, 128)], -1)
    cpack[:, 32:64] = dtb.transpose(1, 0, 2).reshape(128, 32)
    dv = np.stack([inp["D_f"].reshape(NDC, 128), inp["D_b"].reshape(NDC, 128)], -1)
    cpack[:, 64:96] = dv.transpose(1, 0, 2).reshape(128, 32)
    aneg = np.stack([-np.exp(inp["A_log_f"]), -np.exp(inp["A_log_b"])]).astype(f32)
    aneg = aneg.reshape(2, NDC, 128, D_STATE)          # [br, dc, p, n]
    cpack[:, 96:608] = aneg.transpose(2, 0, 1, 3).reshape(128, 512)
    out["opw"] = np.ascontiguousarray(inp["out_proj_w"].T.reshape(NDC, 128, D_MODEL)).astype(f16)
    cpack[:, 616:624] = ln_g.reshape(NMC, 128).T
    cpack[:, 624:632] = ln_b.reshape(NMC, 128).T
    out["cpack"] = np.ascontiguousarray(cpack)
    sel = np.zeros((D_STATE, D_STATE * 128), np.float16)
    for n in range(D_STATE):
        sel[n, n * 128:(n + 1) * 128] = 1.0
    out["sel"] = sel
    return out


def kernel(**inputs):
    inputs = {k: np.asarray(v, dtype=np.float32) for k, v in inputs.items()}
    if "prog" not in _CACHE:
        _CACHE["prog"] = _build_program()
    nc = _CACHE["prog"]

    in_maps = []
    for core in range(8):
        s, b = core // 4, core % 4
        if s == 0:
            x = inputs["g_x"][b]
            rb = (inputs["e_conv_w"], inputs["e_bn_g"], inputs["e_bn_b"], inputs["e_skip_w"])
            lng, lnb = inputs["ln1_g"], inputs["ln1_b"]
        else:
            x = inputs["r_x"][b]
            rb = (inputs["g_conv_w"], inputs["g_bn_g"], inputs["g_bn_b"], inputs["g_skip_w"])
            lng, lnb = inputs["ln2_g"], inputs["ln2_b"]
        in_maps.append(_prep_core_inputs(x, *rb, inputs, lng, lnb))

    trace = bool(os.environ.get("KTRACE"))
    res = None
    for attempt in range(3):
        try:
            res = run_bass_kernel_spmd(nc, in_maps, list(range(8)), trace=trace)
            np.asarray(res.results[0]["y_out"])
            break
        except Exception:
            if attempt == 2:
                raise
            import time
            time.sleep(2.0)
    _CACHE["last_result"] = res
    g_out = np.stack([res.results[b]["y_out"] for b in range(4)]).astype(np.float32)
    r_out = np.stack([res.results[4 + b]["y_out"] for b in range(4)]).astype(np.float32)
    return g_out, r_out
